# revision 20
# baseline (speedup 1.0000x reference)
"""VQ codebook decoder on 8 Trainium2 NeuronCores.

Strategy: the decoder output depends only on which codebook row each token
selects, so decode each *unique* referenced codebook row exactly once and
look tokens up afterwards. The host computes the global unique index set
(np.unique), pre-gathers those rows from the codebook (bf16, already in the
transposed layout mm1 wants), and splits them evenly across the 8 cores
(~U/8 rows each, padded to a multiple of 4). Each core runs the dense
2-layer MLP (1024 -> 4096 gelu -> 1024) over its rows in bf16 with fp32
PSUM accumulation and writes a decoded table; the host gathers per-token
rows from the concatenated tables during unshard (replacing the inverse
permutation scatter a per-token device output would need anyway).

Both matmuls keep weights stationary and stream tokens as the moving
operand, so tensor-engine time scales with the exact row count instead of
rounding up to 128-token tiles. Tokens are processed in PSUM groups
(<=512 each, one bank) per weight block; each group's contraction runs as
consecutive matmuls since retargeting the PSUM bank between matmuls costs
~15ns on HW. Weights stream through small SBUF rings on the
Activation-engine HWDGE queue while the q rows load k-interleaved across
both queues; throwaway warmup matmuls ramp the PE p-state during the DMA
head. gelu+b1 ride the scalar-engine PSUM eviction; the b2 evictions
alternate between vector and scalar so the two groups' tails overlap.
"""

import sys

if "/opt/trn_rl_repo" not in sys.path:
    sys.path.insert(0, "/opt/trn_rl_repo")

import numpy as np
import ml_dtypes

import concourse.mybir as mybir
import concourse.tile as tile
from concourse import bacc
from concourse.bass_utils import run_bass_kernel_spmd

B, M = 32, 576
CB, D, H, O = 8192, 1024, 4096, 1024
N_CORES = 8
P = 128
DK = D // P   # 8   k-subtiles for mm1
HB = H // P   # 32  column blocks of W1 / k-subtiles for mm2
OB = O // P   # 8   column blocks of W2

BF16 = mybir.dt.bfloat16
F32 = mybir.dt.float32
GELU = mybir.ActivationFunctionType.Gelu_apprx_tanh
IDENT = mybir.ActivationFunctionType.Identity

_cache: dict = {}


def _build(uc: int, repeats: int = 1, ngroups: int | None = None,
           psum_bufs: int = 2):
    """MLP over `uc` codebook rows: dec[ob*128+p, t] = rec[t, ob*128+p]."""
    if ngroups is None:
        ngroups = -(-uc // 512)
    gs = []
    rem, left = uc, ngroups
    for _ in range(ngroups):
        g = min(512, -(-rem // left // 4) * 4, rem)
        gs.append(g)
        rem -= g
        left -= 1
    bounds = [0]
    for g in gs:
        bounds.append(bounds[-1] + g)
    assert 0 < uc <= 1024 and uc % 4 == 0 and bounds[-1] == uc
    assert all(0 < g <= 512 for g in gs)

    nc = bacc.Bacc("TRN2", target_bir_lowering=False, debug=False,
                   num_devices=N_CORES)
    # qt[p, k, t] = q[t, k*128+p]; w1h[p, hb, k, c] = W1[k*128+p, hb*128+c]
    # w2h[p, ob, k, c] = W2[k*128+p, ob*128+c]; b1h/b2h column-blocked.
    qt = nc.declare_dram_parameter("qt", [P, DK, uc], BF16, isOutput=False)
    w1h = nc.declare_dram_parameter("w1h", [P, HB, DK * P], BF16,
                                    isOutput=False)
    w2h = nc.declare_dram_parameter("w2h", [P, OB, HB * P], BF16,
                                    isOutput=False)
    b1h = nc.declare_dram_parameter("b1h", [P, HB], F32, isOutput=False)
    b2h = nc.declare_dram_parameter("b2h", [P, OB], F32, isOutput=False)
    dec = nc.declare_dram_parameter("dec", [OB, P, uc], F32, isOutput=True)

    with tile.TileContext(nc) as tc:
        with (
            tc.tile_pool(name="cpool", bufs=1) as cpool,
            tc.tile_pool(name="w1pool", bufs=3) as w1pool,
            tc.tile_pool(name="w2pool", bufs=2) as w2pool,
            tc.tile_pool(name="opool", bufs=2) as opool,
            tc.tile_pool(name="p1pool", bufs=psum_bufs, space="PSUM") as p1pool,
            tc.tile_pool(name="p2pool", bufs=psum_bufs, space="PSUM") as p2pool,
        ):
          for _rep in range(repeats):
            qsb = cpool.tile([P, DK, uc], BF16)
            w1tiles = [w1pool.tile([P, DK, P], BF16, name="w1sb")
                       for _ in range(2)]
            # head: first w1 chunks + q rows k-interleaved across both queues
            nc.scalar.dma_start(out=w1tiles[0][:], in_=w1h[:, 0])
            nc.sync.dma_start(out=qsb[:, 0, :], in_=qt[:, 0, :])
            nc.scalar.dma_start(out=qsb[:, 1, :], in_=qt[:, 1, :])
            nc.sync.dma_start(out=qsb[:, 2, :], in_=qt[:, 2, :])
            nc.scalar.dma_start(out=w1tiles[1][:], in_=w1h[:, 1])
            for k in range(3, DK):
                eng = nc.sync if k % 2 == 0 else nc.scalar
                eng.dma_start(out=qsb[:, k, :], in_=qt[:, k, :])
            if _rep == 0:
                # PE warmup during the DMA head: ramp the p-state on
                # throwaway matmuls so the real stream starts at full clock.
                wsb = cpool.tile([P, 512], BF16, name="warm")
                nc.vector.memset(wsb[:], 0)
                wps = p1pool.tile([P, 512], F32, name="ps0")
                for i in range(8):
                    nc.tensor.matmul(wps[:, 0:512], wsb[:, 0:P],
                                     wsb[:, 0:512], start=(i == 0),
                                     stop=(i == 7))
            b1sb = cpool.tile([P, HB], F32)
            nc.sync.dma_start(out=b1sb[:], in_=b1h[:])
            b2sb = cpool.tile([P, OB], F32)
            nc.sync.dma_start(out=b2sb[:], in_=b2h[:])
            hsb = cpool.tile([P, HB, uc], BF16)

            for hb in range(HB):
                if hb < 2:
                    w1sb = w1tiles[hb]
                else:
                    w1sb = w1pool.tile([P, DK, P], BF16, name="w1sb")
                    nc.scalar.dma_start(out=w1sb[:], in_=w1h[:, hb])
                for g in range(ngroups):
                    ps = p1pool.tile([P, 512], F32, name=f"ps{g}")
                    for k in range(DK):
                        nc.tensor.matmul(ps[:, 0:gs[g]], w1sb[:, k, :],
                                         qsb[:, k, bounds[g]:bounds[g + 1]],
                                         start=(k == 0), stop=(k == DK - 1))
                    nc.scalar.activation(
                        hsb[:, hb, bounds[g]:bounds[g + 1]],
                        ps[:, 0:gs[g]], GELU, bias=b1sb[:, hb:hb + 1])

            for ob in range(OB):
                w2sb = w2pool.tile([P, HB, P], BF16, name="w2sb")
                nc.scalar.dma_start(out=w2sb[:], in_=w2h[:, ob])
                osb = opool.tile([P, uc], F32, name="osb")
                for g in range(ngroups):
                    lo, hi = bounds[g], bounds[g + 1]
                    ps = p2pool.tile([P, 512], F32, name=f"ps2{g % 2}")
                    for k in range(HB):
                        nc.tensor.matmul(ps[:, 0:hi - lo], w2sb[:, k, :],
                                         hsb[:, k, lo:hi],
                                         start=(k == 0), stop=(k == HB - 1))
                    if g % 2 == 0:
                        nc.vector.tensor_scalar_add(osb[:, lo:hi],
                                                    ps[:, 0:hi - lo],
                                                    b2sb[:, ob:ob + 1])
                    else:
                        nc.scalar.activation(osb[:, lo:hi], ps[:, 0:hi - lo],
                                             IDENT, bias=b2sb[:, ob:ob + 1])
                    nc.sync.dma_start(out=dec[ob, :, lo:hi], in_=osb[:, lo:hi])

    nc.compile()
    return nc


def _get_nc(uc: int, repeats: int = 1, ngroups: int | None = None,
            psum_bufs: int = 2):
    key = (uc, repeats, ngroups, psum_bufs)
    if key not in _cache:
        _cache[key] = _build(uc, repeats, ngroups, psum_bufs)
    return _cache[key]


def _plan(index, codebook, W1, b1, W2, b2):
    """Host-side sharding: global unique rows -> per-core transposed tiles."""
    bf = ml_dtypes.bfloat16
    flat = np.asarray(index).reshape(-1)
    uniq, inv = np.unique(flat, return_inverse=True)
    u = uniq.size
    uc = max(64, -(-u // (N_CORES * 4)) * 4)  # per-core rows, 4-aligned
    uniq_pad = np.zeros(N_CORES * uc, dtype=uniq.dtype)
    uniq_pad[:u] = uniq
    rows = np.ascontiguousarray(codebook, dtype=np.float32)[uniq_pad]
    rows = rows.astype(bf)
    # [core, t, k, p] -> [core, p, k, t]
    qt_all = np.ascontiguousarray(
        rows.reshape(N_CORES, uc, DK, P).transpose(0, 3, 2, 1))

    w1h = np.ascontiguousarray(
        W1.astype(bf).reshape(DK, P, HB, P).transpose(1, 2, 0, 3)
        .reshape(P, HB, DK * P))
    w2h = np.ascontiguousarray(
        W2.astype(bf).reshape(HB, P, OB, P).transpose(1, 2, 0, 3)
        .reshape(P, OB, HB * P))
    b1h = np.ascontiguousarray(b1.astype(np.float32).reshape(HB, P).T)
    b2h = np.ascontiguousarray(b2.astype(np.float32).reshape(OB, P).T)
    wmaps = {"w1h": w1h, "w2h": w2h, "b1h": b1h, "b2h": b2h}
    return uc, inv, qt_all, wmaps


def kernel(index, codebook, W1, b1, W2, b2):
    uc, inv, qt_all, wmaps = _plan(index, codebook, W1, b1, W2, b2)
    nc = _get_nc(uc)
    in_maps = [{**wmaps, "qt": qt_all[c]} for c in range(N_CORES)]
    res = run_bass_kernel_spmd(nc, in_maps, list(range(N_CORES)))
    # dec[ob, p, t] = rec[t, ob*128+p] -> [t, o]
    dec_all = np.concatenate(
        [res.results[c]["dec"].transpose(2, 0, 1).reshape(uc, O)
         for c in range(N_CORES)], axis=0)
    return dec_all[inv].reshape(B, M, O).astype(np.float32)
